# revision 13
# baseline (speedup 1.0000x reference)
"""Trainium2 Bass kernel: non-causal multi-head attention.

Full shapes: q,k,v [B=2, H=16, S=2048, D=64] f32 -> out [2, 16, 2048, 64].
Sharding: the 32 (batch, head) pairs are split 4-per-core across 8 cores
(data + head parallel, no cross-core communication).

Host prep: q,k,v are cast to bf16; q,k are regrouped into head-PAIRS
[2, S, 128] per core so the DMA xbar transpose (16x128 tiles, 2-byte
dtype) can load Q^T,K^T directly into SBUF as [128, S] with head A on
partitions 0-63 and head B on 64-127 — no PE transposes at all.

Per-core dataflow (per head pair, q-blocks of 512, k-chunks of 128):
  - V DMA'd straight into vext [128, kc, 65] bf16 (ones in col 64);
    k-chunk (t, j) = K rows {1024t + 8p + j} so V loads are 1KB runs
  - per (q-block, k-chunk):
      ST[k, 2, q]   : both heads' score matmuls, 64-row PE tiles at
                      row 0 / row 64 -> run CONCURRENTLY on hardware
      for most chunks, E = exp(ST/8): one 1024-wide ScalarE exp covers
      both heads; for the chunks in DVE_KC the exp runs on the DVE as
      a two-phase Schraudolph approximation (see below), freeing
      ScalarE — the end-to-end bottleneck engine — of 4/16 of its work
      ACC[65, 2, q] += Vext^T @ E  (row 64 = softmax denominator)
  - ACC copied to SBUF (releases the single PSUM acc buffer) and
    stored raw [65, S]; the host divides by the denominator row and
    transposes during unshard.

DVE offload: e1 = i16(SCH_A*st + SCH_B1F) computed by one DVE
tensor_scalar straight from PSUM; its int16 result IS a bf16 bit
pattern encoding c*exp(st/8)*(1+bow). The phase-shifted variant
e2 = i16(SCH_A*st + SCH_B1F + 64) is just e1 + 64 as an INTEGER add
(round(y)+64 == round(y+64)), a cheap 4x-mode DVE op. One bf16
tensor_tensor add forms e_sum = e1 + e2, which mostly cancels the
Schraudolph 2^f-vs-(1+f) bow; SCH_B1F is chosen so the summed pair's
scale is ~1.0, letting e_sum multiply the PLAIN vext like the exact
chunks (one AV matmul per chunk, same as ScalarE chunks; no scaled
vext copy needed). Max end-to-end rel err 1.1e-2 over all 32 heads
in numpy, vs the 2e-2 gate. Per-chunk DVE cost measured on HW:
1123 (tensor_scalar from PSUM) + 352 (int add, 4x mode) + 535
(bf16 add, 2x mode) = 2.01us, vs 1.04us of ScalarE exp saved; with
ScalarE at 11 exps and DVE at 5 Schraudolphs + the acc copy, both
engines sit at ~11.3-11.4us per 16-chunk window.

Queue placement: all V loads and output stores issue from the Pool
(gpsimd) queue and the first q-transpose from the DVE queue, keeping
the SP queue free for the K/Q transposes whose descriptor generation
gates the pipeline head. The acc->SBUF copy of window w is emitted
after window w+1's first DVE chunk so the DVE's strict FIFO never
head-of-line blocks the Schraudolph stream.
"""
import numpy as np

B, H, S, D = 2, 16, 2048, 64
N_CORES = 8
HPC = (B * H) // N_CORES          # heads per core
NPAIR = HPC // 2                  # head pairs per core
SCALE = 1.0 / float(np.sqrt(D))
NKC = S // 128                    # k-chunks of 128
QSB = 512                         # q-block width (per head, paired in PSUM)
NQSB = S // QSB

# k-chunks whose exp runs on the DVE (two-phase Schraudolph).
DVE_KC = (1, 4, 7, 10, 13)
# Schraudolph constants for bf16 bit patterns: bits = round(A*score + B);
# A folds the 1/sqrt(D) softmax scale and log2(e) into the 7-bit-mantissa
# exponent domain. B is tuned (numpy scan over both round and trunc
# f32->i16 conversion) so that val(bits) + val(bits+64) ~= exp(score/8)
# with scale 1.0 and min-max bow deviation (+-1.5% elementwise).
SCH_A = float(np.log2(np.e) * 128.0 * (1.0 / np.sqrt(D)))
SCH_B1F = 16086.6

ST_BUFS = 3                       # PSUM st tiles (2 banks each)
ACC_BUFS = 1                      # PSUM acc tiles (2 banks each)
DVE_NEED = 4                      # AV lag (chunks) for DVE chunks

_CACHE = {}


def _build(repeat: int = 0):
    """repeat=0: plain body (deliverable). repeat>=1: wrap the whole
    per-core body in a For_i hardware loop for slope timing."""
    import contextlib
    import concourse.bacc as bacc
    import concourse.mybir as mybir
    from concourse import tile

    f32 = mybir.dt.float32
    bf16 = mybir.dt.bfloat16
    i16 = mybir.dt.int16

    nc = bacc.Bacc("TRN2", target_bir_lowering=False, debug=False,
                   num_devices=N_CORES)
    q_d = nc.dram_tensor("q", [NPAIR, S, 2 * D], bf16, kind="ExternalInput")
    k_d = nc.dram_tensor("k", [NPAIR, S, 2 * D], bf16, kind="ExternalInput")
    v_d = nc.dram_tensor("v", [HPC, S, D], bf16, kind="ExternalInput")
    o_d = nc.dram_tensor("outT", [HPC, D + 1, S], f32,
                         kind="ExternalOutput")

    with tile.TileContext(nc) as tc:
        with (
            (tc.For_i(0, repeat) if repeat else contextlib.nullcontext()),
            tc.tile_pool(name="consts", bufs=1) as consts,
            tc.tile_pool(name="trans", bufs=2) as trans,
            tc.tile_pool(name="vex", bufs=2) as vex,
            tc.tile_pool(name="ework", bufs=6) as ework,
            tc.tile_pool(name="norm", bufs=3) as norm,
            tc.tile_pool(name="st", bufs=ST_BUFS, space="PSUM") as st_psum,
            tc.tile_pool(name="acc", bufs=ACC_BUFS, space="PSUM") as acc_psum,
        ):
            ones_bf = consts.tile([128, 1], bf16)
            nc.vector.memset(ones_bf, 1.0)

            # Window-w acc->SBUF copy + store, deferred into window w+1
            # (see module docstring).
            pending_store = []

            def emit_store(acc, pair, q0, final):
                # Ship the raw accumulator (numerator rows 0:64 +
                # denominator row 64); the final divide happens on the
                # host during unshard. The copy to SBUF doubles as the
                # PSUM release (DMA cannot read PSUM).
                accS = norm.tile([D + 1, 2, QSB], f32, tag="accS",
                                 name="accS")
                nchunk = 2 if final else 1
                HQ = QSB // nchunk
                for c in range(nchunk):
                    nc.vector.tensor_copy(
                        accS[:, :, c * HQ:(c + 1) * HQ],
                        acc[:, :, c * HQ:(c + 1) * HQ])
                    nc.sync.dma_start(
                        o_d[pair * 2:pair * 2 + 2, :,
                            q0 + c * HQ:q0 + (c + 1) * HQ]
                        .rearrange("h d s -> d h s"),
                        accS[:, :, c * HQ:(c + 1) * HQ])

            for pair in range(NPAIR):
                # Per-chunk transpose tiles: each [128, 512] chunk is its
                # own tile so the first ST only waits for chunk 0, not the
                # whole [S, 128] transpose.
                NTC = S // QSB
                qTs = [trans.tile([128, QSB], bf16, tag=f"qT{t}",
                                  name=f"qT{t}") for t in range(NTC)]
                kTs = [trans.tile([128, 2 * QSB], bf16, tag=f"kT{t}",
                                  name=f"kT{t}") for t in range(2)]
                # k-chunk (t, j) = K rows {1024t + 8p + j : p=0..127}; the
                # row order within a chunk is irrelevant (summed over), so
                # picking stride-8 columns of kT tile t lets V load as
                # 1KB-contiguous runs per partition, 2 DMAs per tensor.
                vexts = []
                for sub in range(2):
                    vexts.append(vex.tile([128, NKC, D + 1], bf16,
                                          tag=f"vext{sub}",
                                          name=f"vext{sub}"))
                # Queue order follows consumption order: kc 0-7 need kT0 +
                # vext halves 0; kc 8-15 need kT1 + halves 1; qT_t per 16.
                # V loads and stores go via the Pool queue, the very first
                # qT via the DVE queue, so the SP queue reaches kT0's (and
                # later kT1/qT's) descriptor generation immediately.
                nc.sync.dma_start_transpose(
                    kTs[0], k_d[pair][0:2 * QSB, :])
                if pair == 0:
                    # Activation's HWDGE queue is free until the first exp:
                    # qT0's descriptor generation runs in parallel with
                    # kT0's on SP, pulling the first ST ~1.3us earlier.
                    nc.scalar.dma_start_transpose(
                        qTs[0], q_d[pair][0:QSB, :])
                else:
                    nc.sync.dma_start_transpose(
                        qTs[0], q_d[pair][0:QSB, :])
                for t in range(2):
                    if t > 0:
                        nc.sync.dma_start_transpose(
                            kTs[t], k_d[pair][t * 2 * QSB:(t + 1) * 2 * QSB, :])
                    for sub in range(2):
                        h = pair * 2 + sub
                        nc.sync.dma_start(
                            vexts[sub][:, t * 8:(t + 1) * 8, 0:D],
                            v_d[h][t * 2 * QSB:(t + 1) * 2 * QSB].rearrange(
                                "(p j) d -> p j d", p=128, j=8))
                for t in range(1, NTC):
                    nc.sync.dma_start_transpose(
                        qTs[t], q_d[pair][t * QSB:(t + 1) * QSB, :])
                for sub in range(2):
                    nc.vector.tensor_copy(vexts[sub][:, :, D],
                                          ones_bf.broadcast_to([128, NKC]))

                # Both heads of the pair run through the pipeline together:
                # their STs are 64-row PE tiles at row 0 / row 64
                # (tile_position auto-derived), so on hardware they execute
                # concurrently; one 1024-wide exp covers both heads.
                # AV emission lags ST/exp by two k-chunks (with st bufs=3)
                # so the ST feeding exp(n+1) never queues behind an AV that
                # is still waiting on exp(n). DVE chunks get a longer lag
                # (need=4) to cover the slower Schraudolph pair.
                def emit_av(acc, kc, ev):
                    first, last = (kc == 0), (kc == NKC - 1)
                    for sub in range(2):
                        nc.tensor.matmul(
                            acc[:, sub, :],
                            vexts[sub][:, kc, :],
                            ev[1][:, sub, :],
                            start=first, stop=last)

                for qsb in range(S // QSB):
                    q0 = qsb * QSB
                    final = (pair == NPAIR - 1) and (qsb == S // QSB - 1)
                    lag = 0 if final else 2
                    acc = acc_psum.tile([D + 1, 2, QSB], f32, tag="acc")
                    es = {}
                    for kc in range(NKC):
                        st = st_psum.tile([128, 2, QSB], f32, tag="st")
                        t, j = kc // 8, kc % 8
                        for sub in range(2):
                            kstat = kTs[t][sub * D:(sub + 1) * D].rearrange(
                                "d (p8 j) -> d j p8", j=8)[:, j, :]
                            nc.tensor.matmul(
                                st[:, sub, :],
                                kstat,
                                qTs[qsb][sub * D:(sub + 1) * D, :],
                                start=True, stop=True)
                        if kc in DVE_KC:
                            e1 = ework.tile([128, 2, QSB], i16, tag="e1")
                            e2 = ework.tile([128, 2, QSB], i16, tag="e2")
                            esum = ework.tile([128, 2, QSB], bf16,
                                              tag="esum")
                            nc.vector.tensor_scalar(
                                e1, st, SCH_A, SCH_B1F,
                                mybir.AluOpType.mult, mybir.AluOpType.add)
                            nc.vector.tensor_scalar_add(e2, e1, 64.0)
                            nc.vector.tensor_tensor(
                                esum, e1.bitcast(bf16), e2.bitcast(bf16),
                                mybir.AluOpType.add)
                            es[kc] = ("dve", esum)
                        else:
                            e = ework.tile([128, 2, QSB], bf16, tag="e")
                            nc.scalar.activation(
                                e, st, mybir.ActivationFunctionType.Exp,
                                scale=SCALE)
                            es[kc] = ("act", e)
                        if kc == 2 and pending_store:
                            emit_store(*pending_store.pop())
                        for k in sorted(es):
                            need = DVE_NEED if k in DVE_KC else lag
                            if kc - k >= need:
                                emit_av(acc, k, es.pop(k))
                    for kc in sorted(es):
                        emit_av(acc, kc, es.pop(kc))
                    if final:
                        emit_store(acc, pair, q0, final=True)
                    else:
                        pending_store.append((acc, pair, q0, False))
            assert not pending_store

    nc.compile()
    return nc


def get_nc():
    if "nc" not in _CACHE:
        _CACHE["nc"] = _build()
    return _CACHE["nc"]


def shard_inputs(q, k, v):
    """Full [B,H,S,D] f32 -> list of 8 per-core input dicts (bf16).

    q,k are cast to bf16 and regrouped into head pairs [NPAIR, S, 2D]
    (pair p column block = heads 2p, 2p+1 side by side) so the device
    xbar-transpose yields [2D, S] with head A on partitions 0:64 and
    head B on 64:128. v stays [HPC, S, D] bf16.
    """
    import ml_dtypes
    bf16 = ml_dtypes.bfloat16
    qf = np.asarray(q, dtype=np.float32).reshape(B * H, S, D).astype(bf16)
    kf = np.asarray(k, dtype=np.float32).reshape(B * H, S, D).astype(bf16)
    vf = np.asarray(v, dtype=np.float32).reshape(B * H, S, D).astype(bf16)

    def pairup(x):                       # [HPC, S, D] -> [NPAIR, S, 2D]
        return np.ascontiguousarray(
            x.reshape(NPAIR, 2, S, D).transpose(0, 2, 1, 3)
            .reshape(NPAIR, S, 2 * D))

    maps = []
    for c in range(N_CORES):
        sl = slice(c * HPC, (c + 1) * HPC)
        maps.append({
            "q": pairup(qf[sl]),
            "k": pairup(kf[sl]),
            "v": np.ascontiguousarray(vf[sl]),
        })
    return maps


def unshard_outputs(results):
    """List of 8 per-core {'outT': [HPC, D+1, S]} -> full [B, H, S, D].

    Row D of each head is the softmax denominator; the final divide
    happens here on the host.
    """
    out = np.empty((B * H, S, D), dtype=np.float32)
    for c in range(N_CORES):
        oT = np.asarray(results[c]["outT"])          # [HPC, D+1, S]
        norm = oT[:, 0:D, :] / oT[:, D:D + 1, :]
        out[c * HPC:(c + 1) * HPC] = norm.transpose(0, 2, 1)
    return out.reshape(B, H, S, D)


def kernel(q, k, v):
    from concourse.bass_utils import run_bass_kernel_spmd
    nc = get_nc()
    in_maps = shard_inputs(q, k, v)
    res = run_bass_kernel_spmd(nc, in_maps, list(range(N_CORES)))
    return unshard_outputs(res.results)
